# revision 1
# baseline (speedup 1.0000x reference)
"""MinDistanceDecoder (vq_codebook) Trainium2 kernel.

Math: reference computes, per batch row b,
    argmin_w mean_n |llr[b,n] - max_abs * s[w,n]|,   s[w,n] in {+1,-1}
with llr = -4*noisy/sigma2 and max_abs = max|llr|.  Since |llr| <= max_abs
elementwise, |llr - max_abs*s| = max_abs - s*llr exactly, so
    dist[b,w] = max_abs - (1/n) * sum_n s[w,n]*llr[b,n]
and argmin_w dist = argmax_w sum_n s[w,n]*llr[b,n] = argmin_w noisy[b]·s[w]
(llr is a negative scalar multiple of noisy; sigma2 > 0).  The returned value
is possible_words[best] = the LSB-first bit pattern of the argmin index.

Kernel: codebook-sharded across 8 cores (8192 codewords each).  Each core
computes scores'[b,w] = (-noisy[b])·s[w] with bf16 hi/lo x fp8 sign matmuls
(exact to ~1e-5, verified >> the 1.7e-2 top-2 score gap), stacking two
codeword halves on PSUM partitions 0-63 / 64-127, then finds per-chunk
top-8 maxima and their first-occurrence indices with the DVE Max8 /
FindIndex8 instructions.  The host combines the 8 cores x 5 chunks x 2
halves candidates (exact f32 compare, ties to the smallest index) and
decodes the winning index into its bit pattern.

Layout: pair t (t=0..7) scores sT columns [1024t, 1024t+512) on partitions
0-63 and [1024t+512, 1024(t+1)) on partitions 64-127, so one contiguous
sT-column DMA chunk feeds whole pairs; the codebook DMA is split into 5
chunks alternating between the two HWDGE engines (ACT pushes the first
chunk while sync pushes x2) so the first matmul starts as early as
possible, and the first/last DVE chunks are 512 columns so the argmax
starts right after pair 0 and the post-PE tail stays short.
"""

import numpy as np
import ml_dtypes

K = 16
N = 32
B = 64
NW = 2 ** K            # 65536
NCORES = 8
WPC = NW // NCORES     # 8192 codewords per core
HALF = WPC // 2        # 4096 scores columns (x2 partition halves)
# scores-column width per psum pair; narrow leading pairs let the first
# matmul/copy/argmax start as early as possible behind the codebook DMA
PAIR_W = [512, 512, 512, 512, 512, 512, 512, 512]
SC_BASE = [0]
for _w in PAIR_W[:-1]:
    SC_BASE.append(SC_BASE[-1] + _w)
NPAIR = len(PAIR_W)
# DVE argmax chunks (columns of the scores tile), aligned to pair boundaries
DVE_CHUNKS = [512, 1024, 1024, 1024, 512]
DVE_BASES = [0, 512, 1536, 2560, 3584]
NCHUNK = len(DVE_CHUNKS)
# codebook DMA column chunks (sT columns = 2x scores columns)
DMA_CHUNKS = [1024, 1024, 2048, 2048, 2048]
DMA_BASES = [0, 1024, 2048, 4096, 6144]

_CACHE = {}


def _split_excess_waits(nc, mybir, maxw=1):
    """Walrus (core_v3) rejects instructions carrying too many sem waits
    ("Too many sync wait commands") -- split extras onto standalone
    event-semaphore wait instructions placed just before."""
    for f in nc.m.functions:
        for bb in f.blocks:
            new = []
            for ins in bb.instructions:
                si = ins.sync_info
                if si is not None and si.on_wait and len(si.on_wait) > maxw:
                    waits = list(si.on_wait)
                    extra, keep = waits[:-maxw], waits[-maxw:]
                    for j, w in enumerate(extra):
                        sw = mybir.InstEventSemaphore(
                            name=f"{ins.name}-wsplit{j}", ins=[], outs=[],
                            sync_info=mybir.SyncInfo(on_wait=[w], on_update=[]))
                        sw.engine = ins.engine
                        new.append(sw)
                    ins.sync_info = mybir.SyncInfo(
                        on_wait=keep, on_update=list(si.on_update))
                new.append(ins)
            bb.instructions = new


def _build(split_waits=True):
    import concourse.bass as bass
    import concourse.mybir as mybir
    from concourse.tile import TileContext

    nc = bass.Bass()
    sT = nc.dram_tensor("sT", [N, WPC], mybir.dt.float8e4, kind="ExternalInput")
    x2 = nc.dram_tensor("x2", [N, 2 * B], mybir.dt.bfloat16,
                        kind="ExternalInput")   # cols 0-63 hi, 64-127 lo
    out = nc.dram_tensor("out", [128, 2 * NCHUNK], mybir.dt.uint32,
                         kind="ExternalOutput")  # per chunk: top-1 val (f32 bits), idx

    with TileContext(nc) as tc:
        with (
            tc.tile_pool(name="inp", bufs=1) as inp_pool,
            tc.tile_pool(name="cb", bufs=1) as cb_pool,
            tc.tile_pool(name="ps", bufs=4, space="PSUM") as psum_pool,
            tc.tile_pool(name="sc", bufs=1) as sc_pool,
            tc.tile_pool(name="o", bufs=1) as out_pool,
        ):
            xt = inp_pool.tile([N, 2 * B], mybir.dt.bfloat16)
            nc.sync.dma_start(xt[:], x2[:])
            xh = xt[:, 0:B]
            xl = xt[:, B:2 * B]

            st = cb_pool.tile([N, WPC], mybir.dt.float8e4)
            # Codebook (fp8: +/-1 is exact) split over both HWDGE queues;
            # ACT pushes chunk 0's half A while sync pushes x2 then half B,
            # so the first (half-A) matmuls start as soon as possible.
            nc.scalar.dma_start(st[:, 0:512], sT[:, 0:512])
            nc.sync.dma_start(st[:, 512:1024], sT[:, 512:1024])
            dma_engines = [None, nc.sync, nc.scalar, nc.sync, nc.scalar]
            for d in range(1, len(DMA_CHUNKS)):
                cs = slice(DMA_BASES[d], DMA_BASES[d] + DMA_CHUNKS[d])
                dma_engines[d].dma_start(st[:, cs], sT[:, cs])

            # one scores tile per DVE chunk: a single shared tile lets the
            # scheduler's coarse access tracking serialize chunk-k argmax
            # reads against later pairs' PSUM copies (observed 1.6-2.9us
            # Vector idle mid-chain)
            sc_tiles = []
            for k in range(NCHUNK):
                sck = sc_pool.tile([128, DVE_CHUNKS[k]], mybir.dt.float32,
                                   tag=f"sc{k}")
                sc_tiles.append(sck)
            ot = out_pool.tile([128, 16 * NCHUNK], mybir.dt.uint32)

            for t in range(NPAIR):
                w = PAIR_W[t]
                stb = 2 * SC_BASE[t]
                ps = psum_pool.tile([128, 512], mybir.dt.float32)
                ca = slice(stb, stb + w)                     # half A columns
                cb = slice(stb + w, stb + 2 * w)             # half B columns
                nc.tensor.matmul(ps[0:64, :w], xh, st[:, ca],
                                 start=True, stop=False)
                nc.tensor.matmul(ps[0:64, :w], xl, st[:, ca],
                                 start=False, stop=True)
                nc.tensor.matmul(ps[64:128, :w], xh, st[:, cb],
                                 start=True, stop=False)
                nc.tensor.matmul(ps[64:128, :w], xl, st[:, cb],
                                 start=False, stop=True)
                k = max(i for i in range(NCHUNK) if DVE_BASES[i] <= SC_BASE[t])
                off = SC_BASE[t] - DVE_BASES[k]
                nc.scalar.copy(sc_tiles[k][:, off:off + w], ps[:, :w])

            for k in range(NCHUNK):
                ch = sc_tiles[k][:]
                vsl = ot[:, 16 * k:16 * k + 8].bitcast(mybir.dt.float32)
                nc.vector.max(out=vsl, in_=ch)
                nc.vector.max_index(out=ot[:, 16 * k + 8:16 * k + 16],
                                    in_max=vsl, in_values=ch)

            # ship only the top-1 value + index per chunk (cols 0 and 8 of
            # each 16-wide block) -- 8x less output DMA
            nc.sync.dma_start(out[:], ot[:, 0:16 * NCHUNK:8])

    if split_waits:
        _split_excess_waits(nc, mybir)
    return nc


def _get_nc():
    if "nc" not in _CACHE:
        _CACHE["nc"] = _build()
    return _CACHE["nc"]


def _host_codebook_sT(G):
    """sT[n, w] = 1 - 2*((words[w] @ G) % 2), fp8e4m3, [N, NW]."""
    Gb = (np.asarray(G) % 2).astype(np.uint8)
    w_idx = np.arange(NW, dtype=np.uint32)
    bits = ((w_idx[:, None] >> np.arange(K)[None, :]) & 1).astype(np.uint8)
    cw = np.zeros((NW, N), dtype=np.uint8)
    for i in range(K):
        np.bitwise_xor(cw, bits[:, i:i + 1] & Gb[i][None, :], out=cw)
    s = (1.0 - 2.0 * cw.astype(np.float32))
    return np.ascontiguousarray(s.T).astype(ml_dtypes.float8_e4m3), bits


def kernel(noisy_symbols, G, sigma2):
    from concourse.bass_utils import run_bass_kernel_spmd

    noisy = np.asarray(noisy_symbols, dtype=np.float32)
    assert noisy.shape == (B, N)

    # scores' = (-noisy) @ s^T ; maximize.  sigma2 > 0 only scales.
    xT = np.ascontiguousarray((-noisy).T)            # [N, B] f32
    xh32 = xT.astype(ml_dtypes.bfloat16).astype(np.float32)
    x2 = np.concatenate(
        [xh32.astype(ml_dtypes.bfloat16),
         (xT - xh32).astype(ml_dtypes.bfloat16)], axis=1)   # [N, 2B]
    x2 = np.ascontiguousarray(x2)

    sT_full, bits = _host_codebook_sT(G)             # [N, NW] bf16

    in_maps = []
    for c in range(NCORES):
        in_maps.append({
            "sT": np.ascontiguousarray(sT_full[:, c * WPC:(c + 1) * WPC]),
            "x2": x2,
        })

    nc = _get_nc()
    res = run_bass_kernel_spmd(nc, in_maps, list(range(NCORES)))
    _CACHE["last_results"] = res

    # Host combine: per (core, chunk, half) top-1 value + index -> global
    # argmax (ties -> smallest codeword index, matching jnp.argmin).
    # Scores column c in pair t (SC_BASE[t] <= c < SC_BASE[t]+PAIR_W[t])
    # maps to w_local = 2*SC_BASE[t] + PAIR_W[t]*h + (c - SC_BASE[t]);
    # FindIndex8 returns the first occurrence = smallest w_local per half.
    sc_base_arr = np.asarray(SC_BASE + [HALF])
    pw_arr = np.asarray(PAIR_W)
    best_val = np.full((B,), -np.inf, dtype=np.float64)
    best_w = np.zeros((B,), dtype=np.int64)
    p = np.arange(128)
    b_of_p = p % 64
    h_of_p = p // 64
    for c in range(NCORES):
        o = np.asarray(res.results[c]["out"])        # [128, 2*NCHUNK] u32
        for k in range(NCHUNK):
            val = np.ascontiguousarray(o[:, 2 * k]).view(np.float32).astype(np.float64)
            col = DVE_BASES[k] + o[:, 2 * k + 1].astype(np.int64)
            t = np.searchsorted(sc_base_arr, col, side="right") - 1
            w = (c * WPC + 2 * sc_base_arr[t] + pw_arr[t] * h_of_p
                 + (col - sc_base_arr[t]))
            for pp in range(128):
                bb = b_of_p[pp]
                if (val[pp] > best_val[bb]) or (
                        val[pp] == best_val[bb] and w[pp] < best_w[bb]):
                    best_val[bb] = val[pp]
                    best_w[bb] = w[pp]

    return bits[best_w].astype(np.float32)           # [B, K] LSB-first bits



# revision 6
# speedup vs baseline: 1.4281x; 1.4281x over previous
"""MinDistanceDecoder (vq_codebook) Trainium2 kernel, v2.

Math: argmin_w mean_n |llr[b,n] - max_abs*s[w,n]| == argmax_w (-noisy[b])*s[w]
(see v1 docstring for the reduction).  The returned value is
possible_words[best] = the LSB-first bit pattern of the argmin index.

v2 design (vs the v1 hi/lo 2-pass kernel):
- Single bf16-weight matmul pass: weights = (-noisy)^T bf16 [32, 64] loaded
  ONCE, fp8 +/-1 codebook streams through the PE once (16 matmuls x 512
  cols, A-half -> PSUM partitions 0-63, B-half -> 64-127).  Host-side
  verification shows the bf16-weight chain keeps the true argmax's f32
  score 1.2e-3 above the next fp16 rounding bucket (PE f32 accumulation
  noise is ~1e-4), so one pass suffices when the host re-scores a small
  candidate set exactly (below).
- PSUM->SBUF copies convert to fp16 and are split between the ACT engine
  (pairs 0,2,4,6) and GpSimd (1,3,5,7) so the copy stream keeps up with
  the 427ns/pair matmul cadence.  A dummy ACT op up front hoists the
  one-time ACT_TABLE_LOAD (~1.3us) off the critical path.
- Argmax: DVE Max8/FindIndex8 are 1x-rate ops (dtype-independent), so
  scanning all 4096 columns twice costs ~8.7us.  Instead, 7 fp16
  tensor_tensor-max folds (2x mode, 327ns each) reduce the 8 pair tiles
  to one [128, 512] array F[j] = max_t sc_t[j]; one Max8 + FindIndex8
  over 512 columns finds the top-8 fold values + first-occurrence slots.
  fp16 rounding is monotonic, so the true argmax's slot always holds F's
  top-1 value.
- Output: one contiguous [128, 16] u16 DMA (8 fp16 values | 8 u16 slots).
  v1 shipped a stride-8 slice of the out tile, which the DGE exploded
  into 1280 4-byte descriptors (~5.5us of queue drain + teardown stall).
- Host: for each shipped slot j the candidate codewords are
  w = 1024t + 512h + j for t in 0..8 (8 fold positions); the host
  re-scores all candidates exactly in f64 and picks the argmax with
  ties -> smallest w, which reproduces the reference argmin exactly.
"""

import numpy as np
import ml_dtypes

K = 16
N = 32
B = 64
NW = 2 ** K            # 65536
NCORES = 8
WPC = NW // NCORES     # 8192 codewords per core
NPAIR = 8              # 8 psum pairs of 512 score columns x 2 halves
PW = 512               # scores columns per pair (per half)
# codebook DMA column chunks (sT columns)
DMA_CHUNKS = [1024, 1024, 2048, 2048, 2048]
DMA_BASES = [0, 1024, 2048, 4096, 6144]

_CACHE = {}


def _split_excess_waits(nc, mybir, maxw=1):
    """Walrus (core_v3) rejects instructions carrying too many sem waits
    ("Too many sync wait commands") -- split extras onto standalone
    event-semaphore wait instructions placed just before."""
    for f in nc.m.functions:
        for bb in f.blocks:
            new = []
            for ins in bb.instructions:
                si = ins.sync_info
                if si is not None and si.on_wait and len(si.on_wait) > maxw:
                    waits = list(si.on_wait)
                    extra, keep = waits[:-maxw], waits[-maxw:]
                    for j, w in enumerate(extra):
                        sw = mybir.InstEventSemaphore(
                            name=f"{ins.name}-wsplit{j}", ins=[], outs=[],
                            sync_info=mybir.SyncInfo(on_wait=[w], on_update=[]))
                        sw.engine = ins.engine
                        new.append(sw)
                    ins.sync_info = mybir.SyncInfo(
                        on_wait=keep, on_update=list(si.on_update))
                new.append(ins)
            bb.instructions = new


def _build(split_waits=True):
    import concourse.bass as bass
    import concourse.mybir as mybir
    from concourse.tile import TileContext

    nc = bass.Bass()
    sT = nc.dram_tensor("sT", [N, WPC], mybir.dt.float8e4, kind="ExternalInput")
    xh = nc.dram_tensor("xh", [N, B], mybir.dt.bfloat16, kind="ExternalInput")
    out = nc.dram_tensor("out", [128, 16], mybir.dt.uint16,
                         kind="ExternalOutput")  # 8 fp16 vals | 8 u16 slots

    with TileContext(nc) as tc:
        with (
            tc.tile_pool(name="warm", bufs=1) as warm_pool,
            tc.tile_pool(name="inp", bufs=1) as inp_pool,
            tc.tile_pool(name="cb", bufs=1) as cb_pool,
            tc.tile_pool(name="ps", bufs=4, space="PSUM") as psum_pool,
            tc.tile_pool(name="sc", bufs=1) as sc_pool,
            tc.tile_pool(name="o", bufs=1) as out_pool,
        ):
            # dummy ACT op: forces the one-time ACT_TABLE_LOAD to run during
            # the codebook DMA instead of before the first PSUM copy
            wt = warm_pool.tile([1, 8], mybir.dt.float32)
            nc.vector.memset(wt[:], 0.0)
            nc.scalar.copy(wt[0:1, 4:8], wt[0:1, 0:4])

            xt = inp_pool.tile([N, B], mybir.dt.bfloat16)
            nc.sync.dma_start(xt[:], xh[:])

            st = cb_pool.tile([N, WPC], mybir.dt.float8e4)
            # Codebook (fp8: +/-1 is exact) split over both HWDGE queues;
            # ACT pushes chunk 0's first half while sync pushes xh then the
            # second half, so the first matmuls start as soon as possible.
            nc.scalar.dma_start(st[:, 0:512], sT[:, 0:512])
            nc.sync.dma_start(st[:, 512:1024], sT[:, 512:1024])
            dma_engines = [None, nc.sync, nc.scalar, nc.sync, nc.scalar]
            for d in range(1, len(DMA_CHUNKS)):
                cs = slice(DMA_BASES[d], DMA_BASES[d] + DMA_CHUNKS[d])
                dma_engines[d].dma_start(st[:, cs], sT[:, cs])

            # per-pair fp16 score tiles + fold tiles (separate tiles keep the
            # scheduler's access tracking from serializing unrelated steps)
            sc_t = [sc_pool.tile([128, PW], mybir.dt.float16, name=f"sc{t}", tag=f"sc{t}")
                    for t in range(7)]
            f1 = [sc_pool.tile([128, PW], mybir.dt.float16, name=f"f1{u}", tag=f"f1{u}")
                  for u in range(4)]
            f2 = [sc_pool.tile([128, PW], mybir.dt.float16, name=f"f2{v}", tag=f"f2{v}")
                  for v in range(2)]
            f3 = sc_pool.tile([128, PW], mybir.dt.float16, tag="f3")
            ot = out_pool.tile([128, 16], mybir.dt.uint16)

            # GPSIMD can't read PSUM (BIR verifier), so: ACT copies pairs
            # 0-5, DVE copies pair 6, and pair 7 is folded straight out of
            # PSUM by the DVE (TT-max with a PSUM operand, no copy at all)
            mx = mybir.AluOpType.max
            ps_t = []
            for t in range(NPAIR):
                stb = 1024 * t
                ps = psum_pool.tile([128, PW], mybir.dt.float32)
                ps_t.append(ps)
                nc.tensor.matmul(ps[0:64, :], xt[:], st[:, stb:stb + PW],
                                 start=True, stop=True)
                nc.tensor.matmul(ps[64:128, :], xt[:],
                                 st[:, stb + PW:stb + 2 * PW],
                                 start=True, stop=True)
                if t < 6:
                    nc.scalar.copy(sc_t[t][:], ps[:])

            nc.vector.tensor_tensor(f1[0][:], sc_t[0][:], sc_t[1][:], mx)
            nc.vector.tensor_tensor(f1[1][:], sc_t[2][:], sc_t[3][:], mx)
            nc.vector.tensor_copy(sc_t[6][:], ps_t[6][:])
            nc.vector.tensor_tensor(f1[2][:], sc_t[4][:], sc_t[5][:], mx)
            nc.vector.tensor_tensor(f1[3][:], sc_t[6][:], ps_t[7][:], mx)
            for v in range(2):
                nc.vector.tensor_tensor(f2[v][:], f1[2 * v][:],
                                        f1[2 * v + 1][:], mx)
            nc.vector.tensor_tensor(f3[:], f2[0][:], f2[1][:], mx)

            vals = ot[:, 0:8].bitcast(mybir.dt.float16)
            nc.vector.max(out=vals, in_=f3[:])
            nc.vector.max_index(out=ot[:, 8:16], in_max=vals, in_values=f3[:])

            nc.sync.dma_start(out[:], ot[:])

    if split_waits:
        _split_excess_waits(nc, mybir)
    return nc


def _get_nc():
    if "nc" not in _CACHE:
        _CACHE["nc"] = _build()
    return _CACHE["nc"]


def _host_codebook(G):
    """signs s[w, n] = 1-2*((bits(w) @ G) % 2) as fp8 [N, NW] (transposed),
    plus the bit patterns [NW, K]."""
    Gb = (np.asarray(G) % 2).astype(np.uint8)
    w_idx = np.arange(NW, dtype=np.uint32)
    bits = ((w_idx[:, None] >> np.arange(K)[None, :]) & 1).astype(np.uint8)
    cw = np.zeros((NW, N), dtype=np.uint8)
    for i in range(K):
        np.bitwise_xor(cw, bits[:, i:i + 1] & Gb[i][None, :], out=cw)
    s = (1.0 - 2.0 * cw.astype(np.float32))
    return np.ascontiguousarray(s.T).astype(ml_dtypes.float8_e4m3), s, bits


def kernel(noisy_symbols, G, sigma2):
    from concourse.bass_utils import run_bass_kernel_spmd

    noisy = np.asarray(noisy_symbols, dtype=np.float32)
    assert noisy.shape == (B, N)

    # scores = (-noisy) @ s^T ; maximize.  sigma2 > 0 only scales.
    xT = np.ascontiguousarray((-noisy).T)                  # [N, B] f32
    xh = np.ascontiguousarray(xT.astype(ml_dtypes.bfloat16))

    sT_full, s_signs, bits = _host_codebook(G)             # [N, NW] fp8

    in_maps = []
    for c in range(NCORES):
        in_maps.append({
            "sT": np.ascontiguousarray(sT_full[:, c * WPC:(c + 1) * WPC]),
            "xh": xh,
        })

    nc = _get_nc()
    res = run_bass_kernel_spmd(nc, in_maps, list(range(NCORES)))
    _CACHE["last_results"] = res

    # Host combine: each (core, lane p, rank k) ships (fp16 val, slot j).
    # Candidate codewords: w = core*8192 + 1024t + 512h + j for t in 0..8,
    # h = p // 64; batch row b = p % 64.  Re-score candidates exactly and
    # take the argmax, ties -> smallest w (== reference argmin tie-break).
    TOPK = 8
    t_arr = np.arange(NPAIR)
    cand_w = []      # per-batch lists
    cand_b = []
    p = np.arange(128)
    b_of_p = p % 64
    h_of_p = p // 64
    for c in range(NCORES):
        o = np.asarray(res.results[c]["out"])              # [128, 16] u16
        slots = o[:, 8:8 + TOPK].astype(np.int64)          # [128, K]
        # w[p, k, t]
        w = (c * WPC + 1024 * t_arr[None, None, :]
             + 512 * h_of_p[:, None, None] + slots[:, :, None])
        cand_w.append(w.reshape(128, -1))
        cand_b.append(np.broadcast_to(b_of_p[:, None], (128, TOPK * NPAIR)))
    cand_w = np.concatenate(cand_w, 0).ravel()
    cand_b = np.concatenate(cand_b, 0).ravel()

    # exact scores for the unique candidate codewords
    uw, inv = np.unique(cand_w, return_inverse=True)
    su = s_signs[uw]                                       # [U, N] f64-able
    xs = (-noisy).astype(np.float64)                       # [B, N]
    sc = su.astype(np.float64) @ xs.T                      # [U, B]
    vals = sc[inv, cand_b]

    best_w = np.zeros(B, dtype=np.int64)
    order = np.lexsort((cand_w, -vals))                    # by val desc, w asc
    bb = cand_b[order]
    for i in range(B):
        best_w[i] = cand_w[order[np.flatnonzero(bb == i)[0]]]

    return bits[best_w].astype(np.float32)                 # [B, K] LSB-first
